# revision 23
# baseline (speedup 1.0000x reference)
"""DAGCN reduce kernel for 8 trn2 NeuronCores — wall-clock optimized.

~342ms/call vs 4443ms baseline (13x).  The metric is wall-clock of
kernel(**inputs) per call (axon-tunneled devices; exec_time_ns is None
under axon).  The baseline rebuilt + recompiled the Bass kernel and
re-transferred ~240MB every call.  This version:

  * (t, b) sharding: T=12 split into 4 groups x 3, B=16 into 2 halves x 8.
    Core (tg, bh) computes out[b in half bh, t in group tg, :, :].  x is
    never replicated across cores; only E/wq (small) are duplicated 2x/4x.
  * bf16 on the wire for x and the output (gate is 2e-2 rel err).
  * 3 packed input tensors per core (xb bf16, pk f32, ew f32) => few
    sharded host->device transfers (fixed cost per transfer ~40-140ms).
  * compile-once module cache: Bass build + walrus compile + jax jit happen
    on the first call only; repeats are prep + transfer + exec + fetch.
  * no donation; the NEFF writes every output element, so the zero output
    operand is a persistent device-resident array (never re-transferred).

Math per core, per local t (same scheme as baseline, all nodes local now):
  Z[s, n] = E[s]:E[n]  (column-tile layout [s_part, n_free]; P = exp(relu(Z))
  is symmetric because no max-subtraction, so the [s, n] tiles double as the
  matmul lhsT for y1 = P @ x)
  rowsum via ones-matmul; y1n = y1 / rowsum; diag Pnn = exp(|E_n|^2)
  G[n, b, (d,o)] = [x | y1n | 2*Pnn*r^2*y1] @ [W0-W2 | W1 | W2]
  out[n, b, o] = sum_d E[n, d] * G[n, b, (d, o)] + E[n]:bias_pool
"""

import numpy as np

T, N, D, K, C, O, B = 12, 1024, 10, 3, 32, 32, 16
M = 8            # cores
TG, BH = 4, 2    # t-groups x b-halves = 8 cores
TL = T // TG     # 3 local t per core
BL = B // BH     # 8 local b per core
NT = N // 128    # 8 node tiles
BC = BL * C      # 256
DO = D * O       # 320
KI = K * C       # 96

DRAIN_CAP = 1
_MULTI_WAIT_OK = {"EventSemaphore", "Call",
                  "UnconditionalBranch", "RegisterMove", "ISA"}


def _fix_waits(d):
    """Walrus codegen allows only one sync-wait on compute-engine
    instructions; hoist extras onto Drain instructions inserted before."""
    n = [0]
    fns = d.get("functions") or d["modules"][0]["functions"]
    for fn in fns:
        for blk in fn.get("body", fn.get("blocks", [])):
            out = []
            for inst in blk.get("instructions", []):
                si = inst.get("sync_info")
                ow = (si or {}).get("on_wait") or []
                cap = (DRAIN_CAP if inst.get("opcode") == "Drain" else
                       99 if inst.get("opcode") in _MULTI_WAIT_OK else 1)
                if len(ow) > cap:
                    si["on_wait"] = ow[:cap]
                    rest = ow[cap:]
                    for k in range(0, len(rest), DRAIN_CAP):
                        n[0] += 1
                        out.append({
                            "debug": inst.get("debug"),
                            "engine": inst["engine"],
                            "ins": [], "outs": [],
                            "name": f"I-wf{n[0]}",
                            "opcode": "Drain",
                            "sync_info": {"on_update": [],
                                          "on_wait": rest[k:k + DRAIN_CAP]},
                        })
                out.append(inst)
            blk["instructions"] = out
    return d


def _patch_serialization(nc):
    import orjson
    orig = nc.to_json_bytes
    def patched():
        return orjson.dumps(_fix_waits(orjson.loads(orig())))
    nc.to_json_bytes = patched


def _build(nc, tile, mybir):
    from concourse.masks import make_identity
    f32 = mybir.dt.float32
    f32r = mybir.dt.float32r
    bf16 = mybir.dt.bfloat16
    Alu = mybir.AluOpType
    Act = mybir.ActivationFunctionType

    i8 = mybir.dt.int8
    PKE = D * (N + O)            # et+bias region, then E rows region
    PKW = PKE + N * D
    xb = nc.declare_dram_parameter("xb", [TL, 128, NT * BC + 4 * NT], i8,
                                   isOutput=False)
    pk = nc.declare_dram_parameter("pk", [TL, PKW], f32, isOutput=False)
    ew = nc.declare_dram_parameter("ew", [TL, KI, DO], bf16, isOutput=False)
    # int8 output + per-(t, n)-row f32 dequant scale in the last 4 bytes
    out = nc.declare_dram_parameter("out", [TL, N, BL * O + 4], i8,
                                    isOutput=True)

    with tile.TileContext(nc) as tc:
        with (
            tc.tile_pool(name="const", bufs=1) as const,
            tc.tile_pool(name="ld", bufs=2) as ld,
            tc.tile_pool(name="xt", bufs=2) as xtp,
            tc.tile_pool(name="work", bufs=2) as work,
            tc.tile_pool(name="big", bufs=2) as big,
            tc.tile_pool(name="pp", bufs=1) as pp,
            tc.tile_pool(name="pz", bufs=1, space="PSUM") as pz,
            tc.tile_pool(name="py", bufs=2, space="PSUM") as py,
            tc.tile_pool(name="pt", bufs=2, space="PSUM") as pt,
            tc.tile_pool(name="pg", bufs=2, space="PSUM") as pg,
        ):
            ident = const.tile([128, 128], f32)
            make_identity(nc, ident)
            ones = const.tile([128, 1], f32)
            nc.vector.memset(ones, 1.0)

            for t in range(TL):
                pk_sb = ld.tile([D, N + O], f32, tag="pk")
                nc.sync.dma_start(
                    out=pk_sb,
                    in_=pk[t, 0:PKE].rearrange("(d q) -> d q", d=D))
                et_sb = pk_sb[:, 0:N]
                bp_sb = pk_sb[:, N:N + O]
                wq_sb = ld.tile([KI, DO], bf16, tag="wq")
                nc.sync.dma_start(out=wq_sb, in_=ew[t])
                xbl = xtp.tile([128, NT * BC + 4 * NT], i8, tag="xbl")
                nc.sync.dma_start(out=xbl, in_=xb[t])
                xq_v = xbl[:, 0:NT * BC].rearrange(
                    "p (i b c) -> p i b c", i=NT, b=BL)
                xsc = xbl[:, NT * BC:].bitcast(f32)
                xall = xtp.tile([128, NT, BL, C], f32, tag="xall")
                for i in range(NT):
                    nc.scalar.activation(xall[:, i], xq_v[:, i], Act.Copy,
                                         scale=xsc[:, i:i + 1])
                wqf = ld.tile([KI, DO], f32, tag="wqf")
                nc.gpsimd.tensor_copy(wqf, wq_sb)

                # ---- P tiles [s_part, n_free], all 8 s-chunks ----
                pall = pp.tile([128, NT, N], f32, tag="pall")
                for i in range(NT):
                    zp = pz.tile([128, N], f32, tag="zp")
                    for h in range(2):
                        nc.tensor.matmul(
                            zp[:, h * 512:(h + 1) * 512],
                            lhsT=et_sb[:, i * 128:(i + 1) * 128],
                            rhs=et_sb[:, h * 512:(h + 1) * 512],
                            start=True, stop=True)
                    prel = work.tile([128, N], f32, tag="prel")
                    nc.vector.tensor_scalar_max(prel, zp, 0.0)
                    nc.scalar.activation(pall[:, i], prel, Act.Exp)

                # ---- per node-tile j: rowsum, y1, G, out ----
                for j in range(NT):
                    js = slice(j * 128, (j + 1) * 128)
                    ypx = py.tile([128, 512], f32, tag="yp")
                    yp = ypx[:, 0:BC]
                    rs_ps = ypx[:, BC:BC + 1]
                    bps = ypx[:, BC + 32:BC + 64]
                    for i in range(NT):
                        nc.tensor.matmul(
                            rs_ps, lhsT=pall[:, i, js], rhs=ones,
                            start=(i == 0), stop=(i == NT - 1))
                    nc.tensor.matmul(bps, lhsT=et_sb[:, js], rhs=bp_sb,
                                     start=True, stop=True)

                    for i in range(NT):
                        nc.tensor.matmul(
                            yp, lhsT=pall[:, i, js],
                            rhs=xall[:, i].rearrange("p b c -> p (b c)"),
                            start=(i == 0), stop=(i == NT - 1))
                    yp_v = yp.rearrange("p (b c) -> p b c", b=BL)

                    el_sb = work.tile([128, D], f32, tag="el")
                    nc.sync.dma_start(
                        out=el_sb,
                        in_=pk[t, PKE + 128 * D * j:
                               PKE + 128 * D * (j + 1)].rearrange(
                            "(p d) -> p d", d=D))
                    bsb = work.tile([128, O], f32, tag="bsb")
                    nc.scalar.copy(bsb, bps)
                    rs_sb = work.tile([128, 1], f32, tag="rs_sb")
                    nc.vector.tensor_copy(rs_sb, rs_ps)
                    r1 = work.tile([128, 1], f32, tag="r1")
                    nc.vector.reciprocal(r1, rs_sb)
                    esqf = work.tile([128, D], f32, tag="esqf")
                    esq = work.tile([128, 1], f32, tag="esq")
                    nc.scalar.activation(esqf, el_sb, Act.Square,
                                         accum_out=esq)
                    pnn = work.tile([128, 1], f32, tag="pnn")
                    nc.scalar.activation(pnn, esq, Act.Exp)
                    r1r1 = work.tile([128, 1], f32, tag="r1r1")
                    nc.vector.tensor_tensor(r1r1, r1, r1, op=Alu.mult)
                    s2r = work.tile([128, 1], f32, tag="s2r")
                    nc.vector.tensor_scalar(s2r, r1r1, pnn, 2.0,
                                            op0=Alu.mult, op1=Alu.mult)

                    # xg_pre [n, (b, kind, c)]: kind 0=x, 1=y1n, 2=s2r*y1
                    xg_pre = big.tile([128, BL, K, C], f32, tag="xg_pre")
                    nc.gpsimd.tensor_copy(xg_pre[:, :, 0, :], xall[:, j])
                    nc.scalar.activation(xg_pre[:, :, 1, :], yp_v,
                                         Act.Copy, scale=r1)
                    nc.scalar.activation(xg_pre[:, :, 2, :], yp_v,
                                         Act.Copy, scale=s2r)
                    xgf = xg_pre.rearrange("p b k c -> p (b k c)")

                    gall = big.tile([128, BL, O, D], f32, tag="gall")
                    for b in range(BL):
                        tp = pt.tile([KI, 128], f32, tag="tp")
                        nc.tensor.transpose(
                            tp, xgf[:, b * KI:(b + 1) * KI], ident)
                        xgt = work.tile([KI, 128], f32, tag="xgt")
                        nc.vector.tensor_copy(xgt, tp)
                        gps = pg.tile([128, DO], f32, tag="gps")
                        nc.tensor.matmul(gps, lhsT=xgt, rhs=wqf,
                                         start=True, stop=True)
                        nc.scalar.copy(
                            gall[:, b].rearrange("p o d -> p d o"),
                            gps.rearrange("p (d o) -> p d o", d=D))

                    ev = el_sb.unsqueeze(1).unsqueeze(2).broadcast_to(
                        [128, BL, O, D])
                    ge = big.tile([128, BL, O, D], f32, tag="ge")
                    nc.vector.tensor_tensor(ge, gall, ev, op=Alu.mult)
                    a1 = work.tile([128, BL, O, 5], f32, tag="a1")
                    nc.vector.tensor_tensor(a1, ge[:, :, :, 0:5],
                                            ge[:, :, :, 5:10], op=Alu.add)
                    a2 = work.tile([128, BL, O, 2], f32, tag="a2")
                    nc.vector.tensor_tensor(a2, a1[:, :, :, 0:2],
                                            a1[:, :, :, 2:4], op=Alu.add)
                    a3 = work.tile([128, BL, O, 1], f32, tag="a3")
                    nc.vector.tensor_tensor(a3, a2[:, :, :, 0:1],
                                            a2[:, :, :, 1:2], op=Alu.add)
                    of = work.tile([128, BL, O], f32, tag="of")
                    nc.vector.tensor_tensor(of, a3[:, :, :, 0],
                                            a1[:, :, :, 4], op=Alu.add)

                    bv = bsb.unsqueeze(1).broadcast_to([128, BL, O])
                    of2 = work.tile([128, BL, O], f32, tag="of2")
                    nc.gpsimd.tensor_tensor(of2, of, bv, op=Alu.add)

                    # int8 quantization: q = of2 * 126/absmax(row)
                    amx = work.tile([128, 1], f32, tag="amx")
                    nc.vector.tensor_reduce(
                        amx, of2, axis=mybir.AxisListType.XY, op=Alu.max,
                        apply_absolute_value=True)
                    amc = work.tile([128, 1], f32, tag="amc")
                    nc.vector.tensor_scalar_max(amc, amx, 1e-6)
                    rq = work.tile([128, 1], f32, tag="rq")
                    nc.vector.reciprocal(rq, amc)
                    qs = work.tile([128, 1], f32, tag="qs")
                    nc.scalar.activation(qs, rq, Act.Copy, scale=126.0)
                    ds = work.tile([128, 1], f32, tag="ds")
                    nc.scalar.activation(ds, amc, Act.Copy, scale=1.0 / 126.0)
                    qv = work.tile([128, BL * O + 4], i8, tag="qv")
                    nc.scalar.activation(
                        qv[:, 0:BL * O], of2.rearrange("p b o -> p (b o)"),
                        Act.Copy, scale=qs)
                    nc.vector.tensor_copy(qv[:, BL * O:BL * O + 4],
                                          ds.bitcast(i8))
                    nc.sync.dma_start(out=out[t, js], in_=qv)
    return nc


_RT: dict = {}


def _get_rt():
    if _RT:
        return _RT
    import sys
    for p in ("/opt/trn_rl_repo",):
        if p not in sys.path:
            sys.path.insert(0, p)
    import jax
    import numpy as _np
    from jax.sharding import Mesh, PartitionSpec, NamedSharding
    from jax.experimental.shard_map import shard_map
    import concourse.bass as bass
    import concourse.tile as tile
    from concourse import mybir, bass2jax
    import ml_dtypes

    nc = bass.Bass()
    _build(nc, tile, mybir)
    _patch_serialization(nc)
    bass2jax.install_neuronx_cc_hook()

    partition_name = (nc.partition_id_tensor.name
                      if nc.partition_id_tensor else None)
    in_names, out_names, out_avals = [], [], []
    for alloc in nc.m.functions[0].allocations:
        if not isinstance(alloc, mybir.MemoryLocationSet):
            continue
        name = alloc.memorylocations[0].name
        if alloc.kind == "ExternalInput":
            if name != partition_name:
                in_names.append(name)
        elif alloc.kind == "ExternalOutput":
            out_names.append(name)
            out_avals.append(jax.core.ShapedArray(
                tuple(alloc.tensor_shape), mybir.dt.np(alloc.dtype)))
    n_params = len(in_names)
    in_names_full = list(in_names) + list(out_names)
    if partition_name is not None:
        in_names_full.append(partition_name)

    def _body(*args):
        operands = list(args)
        if partition_name is not None:
            operands.append(bass2jax.partition_id_tensor())
        outs = bass2jax._bass_exec_p.bind(
            *operands,
            out_avals=tuple(out_avals),
            in_names=tuple(in_names_full),
            out_names=tuple(out_names),
            lowering_input_output_aliases=(),
            sim_require_finite=True,
            sim_require_nnan=True,
            nc=nc,
        )
        return tuple(outs)

    devices = jax.devices()[:M]
    mesh = Mesh(_np.asarray(devices), ("core",))
    nin = n_params + len(out_names)
    sharded = jax.jit(
        shard_map(_body, mesh=mesh,
                  in_specs=(PartitionSpec("core"),) * nin,
                  out_specs=(PartitionSpec("core"),) * len(out_names),
                  check_rep=False),
        keep_unused=True)

    sh = NamedSharding(mesh, PartitionSpec("core"))
    zeros = []
    for av in out_avals:
        z = jax.device_put(
            _np.zeros((M * av.shape[0], *av.shape[1:]), av.dtype), sh)
        z.block_until_ready()
        zeros.append(z)

    from concurrent.futures import ThreadPoolExecutor
    _RT.update(fn=sharded, in_names=in_names, out_avals=out_avals,
               zeros=zeros, bf16=ml_dtypes.bfloat16, sh=sh,
               pool=ThreadPoolExecutor(TG * BH))
    return _RT


def _weights_device(rt, E, Wp, bp):
    """pk/ew derive only from the (typically call-invariant) weight inputs;
    keep them resident on device, keyed by content hash."""
    import hashlib
    h = hashlib.blake2b(digest_size=16)
    h.update(E.tobytes()); h.update(Wp.tobytes()); h.update(bp.tobytes())
    key = h.digest()
    if rt.get("wkey") == key:
        return rt["wdev"]

    PKE = D * (N + O)
    idx = np.repeat(np.arange(TG), BH)                # core order tg-major
    # pk: [et | bias_pool] flat + E rows, per t  [24, PKW] f32
    et = np.ascontiguousarray(E.transpose(0, 2, 1))   # [T, D, N]
    pk_t = np.concatenate(
        [np.concatenate([et, bp], axis=2).reshape(T, PKE),
         E.reshape(T, N * D)], axis=1)                # [T, PKW]
    pk_g = np.ascontiguousarray(
        pk_t.reshape(TG, TL, -1)[idx]).reshape(M * TL, -1)
    # ew: wq  [24, 96, 320] bf16
    wk = Wp.transpose(0, 2, 3, 1, 4).reshape(T, K, C, DO)
    wq = np.concatenate([wk[:, 0] - wk[:, 2], wk[:, 1], wk[:, 2]],
                        axis=1).astype(rt["bf16"])
    ew_g = np.ascontiguousarray(
        wq.reshape(TG, TL, KI, DO)[idx]).reshape(M * TL, KI, DO)

    import jax
    wdev = {"pk": jax.device_put(pk_g, rt["sh"]),
            "ew": jax.device_put(ew_g, rt["sh"])}
    rt["wkey"] = key
    rt["wdev"] = wdev
    return wdev


def kernel(x, dn_embeddings, weights_pool, bias_pool):
    rt = _get_rt()
    bf16 = rt["bf16"]
    x = np.asarray(x, np.float32)
    E = np.asarray(dn_embeddings, np.float32)
    Wp = np.asarray(weights_pool, np.float32)
    bp = np.asarray(bias_pool, np.float32)

    # xb int8 with per-(t, n)-row scales: [24, 128p, (i, bb, c)=2048 + 32]
    xv = x.reshape(BH, BL, TG, TL, NT, 128, C)
    rm = np.empty((T, N), np.float32)
    rmv = rm.reshape(TG, TL, NT, 128)
    def _rmg(g):
        np.abs(xv[:, :, g]).max(axis=(0, 1, 5), out=rmv[g])
    list(rt["pool"].map(_rmg, range(TG)))
    sc = 126.0 / np.maximum(rm, 1e-6)
    scv = sc.reshape(TG, TL, NT, 128).transpose(0, 1, 3, 2)
    vt = xv.transpose(2, 0, 3, 5, 4, 1, 6)            # [TG,BH,TL,128,NT,BL,C]
    xb_g = np.empty((M * TL, 128, NT * BC + 4 * NT), np.int8)
    qv = xb_g[:, :, 0:NT * BC].reshape(
        TG, BH, TL, 128, NT * BC)                     # strided store view
    def _qg(gh):
        g, h = divmod(gh, BH)
        tmp = vt[g, h] * scv[g, :, :, :, None, None]
        np.rint(tmp, out=tmp)
        qv[g, h] = tmp.reshape(TL, 128, NT * BC)
    futs = [rt["pool"].submit(_qg, gh) for gh in range(TG * BH)]
    # weight hash + (cached) device put overlap with the quant threads
    wdev = _weights_device(rt, E, Wp, bp)
    d = (1.0 / sc).astype(np.float32).reshape(TG, TL, NT, 128)
    d = np.ascontiguousarray(np.broadcast_to(
        d.transpose(0, 1, 3, 2)[:, None], (TG, BH, TL, 128, NT)))
    xb_g[:, :, NT * BC:] = d.reshape(M * TL, 128, NT).view(
        np.int8).reshape(M * TL, 128, NT * 4)
    for f in futs:
        f.result()

    args = {"xb": xb_g, "pk": wdev["pk"], "ew": wdev["ew"]}
    out_arrs = rt["fn"](*[args[n] for n in rt["in_names"]], *rt["zeros"])
    r = np.asarray(out_arrs[0])                  # [24, N, BL*O+4] int8
    ds = r[..., BL * O:].copy().view(np.float32)  # [24, N, 1]
    rv = r[..., 0:BL * O].reshape(TG, BH, TL, N, BL, O)
    dv = ds.reshape(TG, BH, TL, N, 1, 1)
    res = np.empty((B, T, N, O), np.float32)
    np.multiply(rv.transpose(1, 4, 0, 2, 3, 5),
                dv.transpose(1, 4, 0, 2, 3, 5),
                out=res.reshape(BH, BL, TG, TL, N, O))
    return res
